# revision 19
# baseline (speedup 1.0000x reference)
"""Distributed Trainium2 kernel for BCESleepLoss.

loss = mean(weight_c * (softplus(x) - x*t)) + 1e-4 * sum_n sum_j corr_n[j]^2 / norm_n

where corr_n = full cross-correlation of predictions[n,:,1] with predictions[n,:,2]
and norm_n = sqrt(sum(s1^2) * sum(s2^2)).

Sharding: data-parallel over the batch dim N=32 -> 4 samples on each of 8 cores.
Each core emits per-partition partial stats [128, 16] (sum-of-squared-corr,
sum(s1^2), sum(s2^2) per sample, plus per-class BCE partial sums); the host
does the final (tiny) reduction in float64.

Cross-correlation as matmuls: for each sample, with K=128,
  out[m', nu] += A_cols[:, i:i+128].T @ B_sh[:, 128*i+1 : 128*i+129],  i = 0..64
where A_cols[tau, g] = a_pad[128*g + tau] (a zero-padded reshape of s1) and
B_sh[tau, x] = b_pad[tau + x] (128 shifted copies of zero-padded s2).  The
resulting 128x128 PSUM tile holds every correlation lag exactly once (in
scrambled order), so sum(out^2) == sum(corr^2).  Verified against
np.convolve in float64.
"""

import os

import numpy as np

import concourse.bass as bass
import concourse.mybir as mybir
import concourse.tile as tile
from concourse import bacc
from concourse.bass_utils import run_bass_kernel_spmd

# Problem constants (hardcoded; kernel.py must be self-contained).
N_FULL = 32
L = 8192
C = 3
LAMBDA1 = 1.0
LAMBDA2 = 1e-4

N_CORES = 8
NS = N_FULL // N_CORES  # samples per core = 4

K = 128  # partition / tile size
G = L // K  # 64 columns of signal data
NT = G + 1  # 65 accumulating matmuls per sample
A_W = 3 * G  # 192: A_cols width (64 zero | 64 data | 64 zero)
BP_LEN = 8576  # b_pad length = 128*67 (zeros | 8192 data | zeros)
BW = 8328  # B_sh width (matmuls read cols [1, 8321))

F32 = mybir.dt.float32
BF16 = mybir.dt.bfloat16

LAST_RESULT = None  # BassKernelResults of the most recent run (for test.py)
_CACHED_NC = None


FULL_PARTS = ("stage", "norm", "bsh", "mm", "bce")


def _kernel_body(tc, parts=FULL_PARTS):
    nc = tc.nc
    pred = nc.dram_tensor("predictions", [NS, L, C], F32, kind="ExternalInput").ap()
    targ = nc.dram_tensor("targets", [NS, L, C], F32, kind="ExternalInput").ap()
    out = nc.dram_tensor("out", [K, 16], F32, kind="ExternalOutput").ap()

    add = mybir.AluOpType.add
    mult = mybir.AluOpType.mult
    sub = mybir.AluOpType.subtract

    with (
        tc.tile_pool(name="singles", bufs=1) as singles,
        tc.tile_pool(name="sig", bufs=3) as sig,
        tc.tile_pool(name="acols", bufs=2) as acols_pool,
        tc.tile_pool(name="bsh", bufs=2) as bsh_pool,
        tc.tile_pool(name="scr", bufs=2) as scr,
        tc.tile_pool(name="bce", bufs=1) as bce_pool,
        tc.tile_pool(name="psum", bufs=2, space="PSUM") as psum_pool,
        tc.tile_pool(name="dram", bufs=2, space="DRAM") as dram_pool,
    ):
        # Per-partition partial stats, one DMA out at the end.
        # cols 0:4 = sum(c^2) per sample, 4:8 = sum(a^2), 8:12 = sum(b^2),
        # 12:15 = per-class BCE partial sums, 15 = unused (zero).
        stats = singles.tile([K, 16], F32)
        nc.vector.memset(stats[:], 0.0)

        zeros_bf = singles.tile([K, BP_LEN // K], BF16)
        nc.vector.memset(zeros_bf[:], 0.0)

        # ---- cross-correlation term, one sample at a time ----
        for n in range(NS if "stage" in parts else 0):
            a_f32 = sig.tile([K, G], F32, tag="a_f32")
            b_f32 = sig.tile([K, G], F32, tag="b_f32")
            # a laid out partition-minor [tau, g] = a[128*g + tau] (feeds A_cols);
            # b row-major [p, g] = b[64*p + g] (feeds the contiguous b_pad write).
            nc.gpsimd.dma_start(
                out=a_f32[:], in_=pred[n, :, 1].rearrange("(g p) -> p g", p=K)
            )
            nc.gpsimd.dma_start(
                out=b_f32[:], in_=pred[n, :, 2].rearrange("(p g) -> p g", p=K)
            )

            # norms: sum(a^2), sum(b^2) -> stats (plain mul + reduce; the
            # fancier fused/accum ops (InstISA) fail at runtime here)
            if "norm" in parts:
                scr_g = scr.tile([K, G], F32, tag="scr_g")
                nc.vector.tensor_mul(scr_g[:], a_f32[:], a_f32[:])
                nc.vector.reduce_sum(
                    stats[:, 4 + n : 5 + n], scr_g[:], axis=mybir.AxisListType.X
                )
                scr_g2 = scr.tile([K, G], F32, tag="scr_g")
                nc.vector.tensor_mul(scr_g2[:], b_f32[:], b_f32[:])
                nc.vector.reduce_sum(
                    stats[:, 8 + n : 9 + n], scr_g2[:], axis=mybir.AxisListType.X
                )

            # A_cols: [64 zero cols | a (bf16) | 64 zero cols]
            a_cols = acols_pool.tile([K, A_W], BF16)
            nc.vector.memset(a_cols[:], 0.0)
            nc.vector.tensor_copy(out=a_cols[:, G : 2 * G], in_=a_f32[:])

            # b -> bf16 -> zero-padded DRAM scratch
            if "bsh" not in parts:
                continue
            b_bf = sig.tile([K, G], BF16, tag="b_bf")
            nc.vector.tensor_copy(out=b_bf[:], in_=b_f32[:])
            b_pad = dram_pool.tile([BP_LEN], BF16)
            nc.gpsimd.dma_start(
                out=b_pad[:].rearrange("(p g) -> p g", p=K), in_=zeros_bf[:]
            )
            # b_pad[128 + 64p + g] = b_bf[p, g]: 128 contiguous 128B writes
            nc.gpsimd.dma_start(
                out=b_pad[K : K + L].rearrange("(p g) -> p g", p=K), in_=b_bf[:]
            )

            # B_sh[tau, x] = b_pad[tau + x]: 128 shifted copies (overlapping read)
            b_sh = bsh_pool.tile([K, BW], BF16)
            bp = b_pad[:]
            src = bass.AP(tensor=bp.tensor, offset=bp.offset, ap=[[1, K], [1, BW]])
            nc.gpsimd.dma_start(out=b_sh[:], in_=src)

            if "mm" not in parts:
                continue
            # 65 accumulating matmuls: psum holds every corr lag exactly once
            psum = psum_pool.tile([K, K], F32)
            for i in range(NT):
                nc.tensor.matmul(
                    psum[:],
                    a_cols[:, i : i + K],
                    b_sh[:, K * i + 1 : K * i + 1 + K],
                    start=(i == 0),
                    stop=(i == NT - 1),
                )

            # sum of squared correlation values -> stats.  (DVE cannot read
            # two PSUM operands, so square on ScalarE, reduce on DVE.)
            scr_c2 = scr.tile([K, K], F32, tag="scr_c2")
            nc.scalar.activation(
                out=scr_c2[:], in_=psum[:],
                func=mybir.ActivationFunctionType.Square,
            )
            nc.vector.reduce_sum(
                stats[:, n : n + 1], scr_c2[:], axis=mybir.AxisListType.X
            )

        if "bce" not in parts:
            nc.sync.dma_start(out=out[:], in_=stats[:])
            return
        # ---- BCE term: relu(x) - x*t + ln(1 + exp(-|x|)), per-class sums ----
        FW = NS * L * C // K  # 768
        x_sb = bce_pool.tile([K, FW], F32)
        t_sb = bce_pool.tile([K, FW], F32)
        nc.sync.dma_start(
            out=x_sb[:],
            in_=pred.rearrange("n l c -> (n l c)").rearrange("(p f) -> p f", p=K),
        )
        nc.sync.dma_start(
            out=t_sb[:],
            in_=targ.rearrange("n l c -> (n l c)").rearrange("(p f) -> p f", p=K),
        )
        # sp = ln(1 + exp(-|x|)) via three ACT passes (no Softplus table)
        ax = bce_pool.tile([K, FW], F32)
        nc.scalar.activation(ax[:], x_sb[:], mybir.ActivationFunctionType.Abs)
        ex = bce_pool.tile([K, FW], F32)
        nc.scalar.activation(
            ex[:], ax[:], mybir.ActivationFunctionType.Exp, scale=-1.0
        )
        sp = bce_pool.tile([K, FW], F32)
        nc.scalar.activation(
            sp[:], ex[:], mybir.ActivationFunctionType.Ln, bias=1.0
        )
        # v = relu(x) - x*t + sp, then per-class reduction (class = f % 3)
        rx = bce_pool.tile([K, FW], F32)
        nc.vector.tensor_scalar_max(rx[:], x_sb[:], 0.0)
        xt = bce_pool.tile([K, FW], F32)
        nc.vector.tensor_mul(xt[:], x_sb[:], t_sb[:])
        v = bce_pool.tile([K, FW], F32)
        nc.vector.tensor_sub(v[:], rx[:], xt[:])
        nc.vector.tensor_add(v[:], v[:], sp[:])
        v_view = v[:].rearrange("p (t c) -> p c t", c=C)
        nc.vector.reduce_sum(
            stats[:, 12 : 12 + C], v_view, axis=mybir.AxisListType.X
        )

        nc.sync.dma_start(out=out[:], in_=stats[:])


def _build(parts=FULL_PARTS):
    global _CACHED_NC
    if _CACHED_NC is not None and _CACHED_NC[0] == parts:
        return _CACHED_NC[1]
    nc = bacc.Bacc(
        "TRN2",
        target_bir_lowering=False,
        debug=False,
        enable_asserts=False,
        num_devices=N_CORES,
    )
    with tile.TileContext(nc) as tc:
        _kernel_body(tc, parts)
    nc.compile()
    _CACHED_NC = (parts, nc)
    return nc


def host_reduce(stats_list, weight):
    """Final scalar reduction over per-core [128, 16] stats, in float64."""
    w = np.asarray(weight, dtype=np.float64)
    bce_sum = 0.0
    prox = 0.0
    for stats in stats_list:
        s = np.asarray(stats, dtype=np.float64)
        ss = s[:, 0:4].sum(axis=0)
        sa = s[:, 4:8].sum(axis=0)
        sb = s[:, 8:12].sum(axis=0)
        prox += float((ss / np.sqrt(sa * sb)).sum())
        bce_sum += float((s[:, 12:15].sum(axis=0) * w).sum())
    loss = LAMBDA1 * bce_sum / (N_FULL * L * C) + LAMBDA2 * prox
    return np.float32(loss)


def kernel(predictions, targets, weight, trace=False):
    global LAST_RESULT
    predictions = np.ascontiguousarray(np.asarray(predictions, dtype=np.float32))
    targets = np.ascontiguousarray(np.asarray(targets, dtype=np.float32))
    weight = np.asarray(weight, dtype=np.float32)
    assert predictions.shape == (N_FULL, L, C), predictions.shape

    nc = _build()
    in_maps = [
        {
            "predictions": np.ascontiguousarray(predictions[k * NS : (k + 1) * NS]),
            "targets": np.ascontiguousarray(targets[k * NS : (k + 1) * NS]),
        }
        for k in range(N_CORES)
    ]
    LAST_RESULT = run_bass_kernel_spmd(
        nc, in_maps, core_ids=list(range(N_CORES)), trace=trace
    )
    stats_list = [r["out"] for r in LAST_RESULT.results]
    return host_reduce(stats_list, weight)


# revision 22
# speedup vs baseline: 2.1638x; 2.1638x over previous
"""Distributed Trainium2 kernel for BCESleepLoss.

loss = mean(weight_c * (softplus(x) - x*t)) + 1e-4 * sum_n sum_j corr_n[j]^2 / norm_n

where corr_n = full cross-correlation of predictions[n,:,1] with predictions[n,:,2]
and norm_n = sqrt(sum(s1^2) * sum(s2^2)).

Sharding: data-parallel over the batch dim N=32 -> 4 samples on each of 8 cores.
Each core emits per-partition partial stats [128, 16]; the host does the final
(tiny) reduction in float64.

Cross-correlation as matmuls: for each sample, with K=128,
  out[m', nu] += A_cols[:, i:i+128].T @ B_sh[:, 128*i : 128*i+128],  i = 0..64
where A_cols[tau, g] = a_pad[128*g + tau] (zero-padded reshape of s1, built
on-chip via PE transposes) and B_sh[tau, x] = b_pad[tau + x + 1] (128 shifted
copies of zero-padded s2, staged through a DRAM scratch so a single
overlapping-read DMA can build it).  The 128x128 PSUM tile then holds every
correlation lag exactly once (scrambled), so sum(out^2) == sum(corr^2).
Verified against np.convolve in float64.

All DRAM traffic is contiguous or chunky (the fine-grained stride-3 gathers
are de-strided on-chip); fused/accum InstISA ops are avoided (they fail at
runtime in this environment).
"""

import numpy as np

import concourse.bass as bass
import concourse.mybir as mybir
import concourse.tile as tile
from concourse import bacc
from concourse.bass_utils import run_bass_kernel_spmd
from concourse.masks import make_identity

# Problem constants (hardcoded; kernel.py must be self-contained).
N_FULL = 32
L = 8192
C = 3
LAMBDA1 = 1.0
LAMBDA2 = 1e-4

N_CORES = 8
NS = N_FULL // N_CORES  # samples per core = 4

K = 128  # partition / tile size
G = L // K  # 64 columns of signal data per sample
NT = G + 1  # 65 accumulating matmuls per sample
A_W = 3 * G  # 192: A_cols width (64 zero | 64 data | 64 zero)
BP_LEN = 8576  # b_pad length = 128*67 (zeros | 8192 data | zeros)
BW = 8328  # B_sh width (matmuls read cols [0, 8320))
TPS = L // K  # 64: t-steps per partition in the de-strided [128, 256] layout

F32 = mybir.dt.float32
BF16 = mybir.dt.bfloat16

LAST_RESULT = None  # BassKernelResults of the most recent run (for test.py)
_CACHED_NC = None

FULL_PARTS = ("corr", "bce")


def _kernel_body(tc, parts=FULL_PARTS):
    nc = tc.nc
    pred = nc.dram_tensor("predictions", [NS, L, C], F32, kind="ExternalInput").ap()
    targ = nc.dram_tensor("targets", [NS, L, C], F32, kind="ExternalInput").ap()
    out = nc.dram_tensor("out", [K, 16], F32, kind="ExternalOutput").ap()

    FW = NS * L * C // K  # 768 cols in the flat [128, 768] input layout
    SW = NS * L // K  # 256 cols per de-strided signal

    with (
        tc.tile_pool(name="singles", bufs=1) as singles,
        tc.tile_pool(name="acols", bufs=2) as acols_pool,
        tc.tile_pool(name="bsh", bufs=2) as bsh_pool,
        tc.tile_pool(name="scr", bufs=2) as scr,
        tc.tile_pool(name="bce", bufs=1) as bce_pool,
        tc.tile_pool(name="psum", bufs=2, space="PSUM") as psum_pool,
        tc.tile_pool(name="psumt", bufs=1, space="PSUM") as psumt_pool,
        tc.tile_pool(name="dram", bufs=1, space="DRAM") as dram_pool,
    ):
        # Per-partition partial stats, one DMA out at the end.
        # cols 0:4 = sum(c^2) per sample; col 4 = sum(s1^2), col 5 = sum(s2^2)
        # (per-partition, sample = p // 32); cols 6:9 = per-class BCE sums.
        stats = singles.tile([K, 16], F32)
        nc.vector.memset(stats[:], 0.0)

        # Contiguous input loads, shared by both loss terms.
        # x_sb[p, f] = pred_flat[768*p + f]; partition p holds sample p // 32.
        x_sb = bce_pool.tile([K, FW], F32)
        nc.sync.dma_start(
            out=x_sb[:],
            in_=pred.rearrange("n l c -> (n l c)").rearrange("(p f) -> p f", p=K),
        )
        x_v = x_sb[:].rearrange("p (t c) -> p c t", c=C)

        if "corr" in parts:
            zeros_bf = singles.tile([K, BP_LEN // K], BF16)
            nc.vector.memset(zeros_bf[:], 0.0)
            ident = singles.tile([K, K], BF16)
            make_identity(nc, ident[:])

            # De-stride s1/s2 (stride-3 SBUF reads on DVE) + cast to bf16:
            # a_de[p, u] = s1[p//32][256*(p%32) + u]
            a_de = singles.tile([K, SW], BF16)
            nc.vector.tensor_copy(out=a_de[:], in_=x_v[:, 1, :])
            b_de = singles.tile([K, SW], BF16)
            nc.vector.tensor_copy(out=b_de[:], in_=x_v[:, 2, :])

            # norms in f32 from x_sb: per-partition partials (sample = p//32)
            scr_n = scr.tile([K, SW], F32, tag="scr_n")
            nc.vector.tensor_mul(scr_n[:], x_v[:, 1, :], x_v[:, 1, :])
            nc.vector.reduce_sum(stats[:, 4:5], scr_n[:], axis=mybir.AxisListType.X)
            scr_n2 = scr.tile([K, SW], F32, tag="scr_n")
            nc.vector.tensor_mul(scr_n2[:], x_v[:, 2, :], x_v[:, 2, :])
            nc.vector.reduce_sum(stats[:, 5:6], scr_n2[:], axis=mybir.AxisListType.X)

            # Transpose a_de halves once for ALL samples:
            # a_deT_*[tau, p] = a_de[p, tau (+128)] -> sample p//32, col block p%32
            a_te = psumt_pool.tile([K, K], BF16, tag="a_te")
            nc.tensor.transpose(a_te[:], a_de[:, 0:K], ident[:])
            a_to = psumt_pool.tile([K, K], BF16, tag="a_to")
            nc.tensor.transpose(a_to[:], a_de[:, K : 2 * K], ident[:])

            # Two zeroed DRAM scratch buffers for b_pad (zero regions written once)
            b_pads = [dram_pool.tile([BP_LEN], BF16, tag=f"bp{j}", name=f"b_pad{j}") for j in range(2)]
            for bp in b_pads:
                nc.sync.dma_start(
                    out=bp[:].rearrange("(p g) -> p g", p=K), in_=zeros_bf[:]
                )

            for n in range(NS):
                # A_cols: [64 zero | a bf16 | 64 zero]; even/odd g columns come
                # from the two transpose halves.  a_odd = A_cols shifted one
                # column so every matmul weight slice is 4-byte aligned.
                a_cols = acols_pool.tile([K, A_W], BF16, tag="a_cols")
                nc.vector.memset(a_cols[:], 0.0)
                av = a_cols[:].rearrange("p (g two) -> p two g", two=2)
                nc.vector.tensor_copy(out=av[:, 0, 32:64], in_=a_te[:, 32 * n : 32 * n + 32])
                nc.vector.tensor_copy(out=av[:, 1, 32:64], in_=a_to[:, 32 * n : 32 * n + 32])
                a_odd = acols_pool.tile([K, A_W], BF16, tag="a_odd")
                nc.vector.tensor_copy(out=a_odd[:, 0 : A_W - 1], in_=a_cols[:, 1:A_W])

                # b_pad[128 + 256*p' + u] = b_de[32n+p', u]: contiguous writes
                bp = b_pads[n % 2]
                nc.sync.dma_start(
                    out=bp[K : K + L].rearrange("(p u) -> p u", p=32),
                    in_=b_de[32 * n : 32 * n + 32, :],
                )

                # B_sh[tau, x] = b_pad[tau + x + 1]: one overlapping-read DMA
                b_sh = bsh_pool.tile([K, BW], BF16)
                bpa = bp[:]
                src = bass.AP(
                    tensor=bpa.tensor, offset=bpa.offset + 1, ap=[[1, K], [1, BW]]
                )
                nc.gpsimd.dma_start(out=b_sh[:], in_=src)

                # 65 accumulating matmuls; psum holds every corr lag once
                psum = psum_pool.tile([K, K], F32)
                for i in range(NT):
                    lhsT = (
                        a_cols[:, i : i + K]
                        if i % 2 == 0
                        else a_odd[:, i - 1 : i - 1 + K]
                    )
                    nc.tensor.matmul(
                        psum[:],
                        lhsT,
                        b_sh[:, K * i : K * i + K],
                        start=(i == 0),
                        stop=(i == NT - 1),
                    )

                # sum(c^2) -> stats col n (square on ScalarE, reduce on DVE)
                scr_c2 = scr.tile([K, K], F32, tag="scr_c2")
                nc.scalar.activation(
                    out=scr_c2[:], in_=psum[:],
                    func=mybir.ActivationFunctionType.Square,
                )
                nc.vector.reduce_sum(
                    stats[:, n : n + 1], scr_c2[:], axis=mybir.AxisListType.X
                )

        if "bce" in parts:
            # ---- BCE: relu(x) - x*t + ln(1 + exp(-|x|)), per-class sums ----
            t_sb = bce_pool.tile([K, FW], F32)
            nc.sync.dma_start(
                out=t_sb[:],
                in_=targ.rearrange("n l c -> (n l c)").rearrange("(p f) -> p f", p=K),
            )
            ax = bce_pool.tile([K, FW], F32)
            nc.scalar.activation(ax[:], x_sb[:], mybir.ActivationFunctionType.Abs)
            ex = bce_pool.tile([K, FW], F32)
            nc.scalar.activation(
                ex[:], ax[:], mybir.ActivationFunctionType.Exp, scale=-1.0
            )
            sp = bce_pool.tile([K, FW], F32)
            nc.scalar.activation(
                sp[:], ex[:], mybir.ActivationFunctionType.Ln, bias=1.0
            )
            rx = bce_pool.tile([K, FW], F32)
            nc.vector.tensor_scalar_max(rx[:], x_sb[:], 0.0)
            xt = bce_pool.tile([K, FW], F32)
            nc.vector.tensor_mul(xt[:], x_sb[:], t_sb[:])
            v = bce_pool.tile([K, FW], F32)
            nc.vector.tensor_sub(v[:], rx[:], xt[:])
            nc.vector.tensor_add(v[:], v[:], sp[:])
            v_view = v[:].rearrange("p (t c) -> p c t", c=C)
            nc.vector.reduce_sum(
                stats[:, 6 : 6 + C], v_view, axis=mybir.AxisListType.X
            )

        nc.sync.dma_start(out=out[:], in_=stats[:])


def _build(parts=FULL_PARTS):
    global _CACHED_NC
    if _CACHED_NC is not None and _CACHED_NC[0] == parts:
        return _CACHED_NC[1]
    nc = bacc.Bacc(
        "TRN2",
        target_bir_lowering=False,
        debug=False,
        enable_asserts=False,
        num_devices=N_CORES,
    )
    with tile.TileContext(nc) as tc:
        _kernel_body(tc, parts)
    nc.compile()
    _CACHED_NC = (parts, nc)
    return nc


def host_reduce(stats_list, weight):
    """Final scalar reduction over per-core [128, 16] stats, in float64."""
    w = np.asarray(weight, dtype=np.float64)
    bce_sum = 0.0
    prox = 0.0
    for stats in stats_list:
        s = np.asarray(stats, dtype=np.float64)
        ss = s[:, 0:4].sum(axis=0)
        sa = s[:, 4].reshape(NS, 32).sum(axis=1)
        sb = s[:, 5].reshape(NS, 32).sum(axis=1)
        prox += float((ss / np.sqrt(sa * sb)).sum())
        bce_sum += float((s[:, 6:9].sum(axis=0) * w).sum())
    loss = LAMBDA1 * bce_sum / (N_FULL * L * C) + LAMBDA2 * prox
    return np.float32(loss)


def kernel(predictions, targets, weight, trace=False):
    global LAST_RESULT
    predictions = np.ascontiguousarray(np.asarray(predictions, dtype=np.float32))
    targets = np.ascontiguousarray(np.asarray(targets, dtype=np.float32))
    weight = np.asarray(weight, dtype=np.float32)
    assert predictions.shape == (N_FULL, L, C), predictions.shape

    nc = _build()
    in_maps = [
        {
            "predictions": np.ascontiguousarray(predictions[k * NS : (k + 1) * NS]),
            "targets": np.ascontiguousarray(targets[k * NS : (k + 1) * NS]),
        }
        for k in range(N_CORES)
    ]
    LAST_RESULT = run_bass_kernel_spmd(
        nc, in_maps, core_ids=list(range(N_CORES)), trace=trace
    )
    stats_list = [r["out"] for r in LAST_RESULT.results]
    return host_reduce(stats_list, weight)


# revision 23
# speedup vs baseline: 2.4119x; 1.1147x over previous
"""Distributed Trainium2 kernel for BCESleepLoss.

loss = mean(weight_c * (softplus(x) - x*t)) + 1e-4 * sum_n sum_j corr_n[j]^2 / norm_n

where corr_n = full cross-correlation of predictions[n,:,1] with predictions[n,:,2]
and norm_n = sqrt(sum(s1^2) * sum(s2^2)).

Sharding: data-parallel over the batch dim N=32 -> 4 samples on each of 8 cores.
Each core emits per-partition partial stats [128, 16]; the host does the final
(tiny) reduction in float64.

Cross-correlation as matmuls: for each sample, with K=128,
  out[m', nu] += A_cols[:, i:i+128].T @ B_sh[:, 128*i : 128*i+128],  i = 0..64
where A_cols[tau, g] = a_pad[128*g + tau] (zero-padded reshape of s1, built
on-chip via PE transposes) and B_sh[tau, x] = b_pad[tau + x + 1] (128 shifted
copies of zero-padded s2, staged through a DRAM scratch so a single
overlapping-read DMA can build it).  The 128x128 PSUM tile then holds every
correlation lag exactly once (scrambled), so sum(out^2) == sum(corr^2).
Verified against np.convolve in float64.

All DRAM traffic is contiguous or chunky (the fine-grained stride-3 gathers
are de-strided on-chip); fused/accum InstISA ops are avoided (they fail at
runtime in this environment).
"""

import numpy as np

import concourse.bass as bass
import concourse.mybir as mybir
import concourse.tile as tile
from concourse import bacc
from concourse.bass_utils import run_bass_kernel_spmd
from concourse.masks import make_identity

# Problem constants (hardcoded; kernel.py must be self-contained).
N_FULL = 32
L = 8192
C = 3
LAMBDA1 = 1.0
LAMBDA2 = 1e-4

N_CORES = 8
NS = N_FULL // N_CORES  # samples per core = 4

K = 128  # partition / tile size
G = L // K  # 64 columns of signal data per sample
NT = G + 1  # 65 accumulating matmuls per sample
A_W = 3 * G  # 192: A_cols width (64 zero | 64 data | 64 zero)
BP_LEN = 8576  # b_pad length = 128*67 (zeros | 8192 data | zeros)
BW = 8328  # B_sh width (matmuls read cols [0, 8320))
TPS = L // K  # 64: t-steps per partition in the de-strided [128, 256] layout

F32 = mybir.dt.float32
BF16 = mybir.dt.bfloat16

LAST_RESULT = None  # BassKernelResults of the most recent run (for test.py)
_CACHED_NC = None

FULL_PARTS = ("corr", "bce")


def _kernel_body(tc, parts=FULL_PARTS):
    nc = tc.nc
    pred = nc.dram_tensor("predictions", [NS, L, C], F32, kind="ExternalInput").ap()
    targ = nc.dram_tensor("targets", [NS, L, C], F32, kind="ExternalInput").ap()
    out = nc.dram_tensor("out", [K, 16], F32, kind="ExternalOutput").ap()

    FW = NS * L * C // K  # 768 cols in the flat [128, 768] input layout
    SW = NS * L // K  # 256 cols per de-strided signal

    with (
        tc.tile_pool(name="singles", bufs=1) as singles,
        tc.tile_pool(name="acols", bufs=2) as acols_pool,
        tc.tile_pool(name="bsh", bufs=3) as bsh_pool,
        tc.tile_pool(name="scr", bufs=2) as scr,
        tc.tile_pool(name="bce", bufs=1) as bce_pool,
        tc.tile_pool(name="psum", bufs=2, space="PSUM") as psum_pool,
        tc.tile_pool(name="psumt", bufs=1, space="PSUM") as psumt_pool,
        tc.tile_pool(name="dram", bufs=1, space="DRAM") as dram_pool,
    ):
        # Per-partition partial stats, one DMA out at the end.
        # cols 0:4 = sum(c^2) per sample; col 4 = sum(s1^2), col 5 = sum(s2^2)
        # (per-partition, sample = p // 32); cols 6:9 = per-class BCE sums.
        stats = singles.tile([K, 16], F32)
        nc.vector.memset(stats[:], 0.0)

        # Contiguous input loads, shared by both loss terms.
        # x_sb[p, f] = pred_flat[768*p + f]; partition p holds sample p // 32.
        x_sb = bce_pool.tile([K, FW], F32)
        nc.sync.dma_start(
            out=x_sb[:],
            in_=pred.rearrange("n l c -> (n l c)").rearrange("(p f) -> p f", p=K),
        )
        x_v = x_sb[:].rearrange("p (t c) -> p c t", c=C)

        if "corr" in parts:
            zeros_bf = singles.tile([K, BP_LEN // K], BF16)
            nc.vector.memset(zeros_bf[:], 0.0)
            ident = singles.tile([K, K], BF16)
            make_identity(nc, ident[:])

            # De-stride s1/s2 (stride-3 SBUF reads on DVE) + cast to bf16:
            # a_de[p, u] = s1[p//32][256*(p%32) + u]
            a_de = singles.tile([K, SW], BF16)
            nc.vector.tensor_copy(out=a_de[:], in_=x_v[:, 1, :])
            b_de = singles.tile([K, SW], BF16)
            nc.vector.tensor_copy(out=b_de[:], in_=x_v[:, 2, :])

            # norms in f32 from x_sb: per-partition partials (sample = p//32)
            scr_n = scr.tile([K, SW], F32, tag="scr_n")
            nc.vector.tensor_mul(scr_n[:], x_v[:, 1, :], x_v[:, 1, :])
            nc.vector.reduce_sum(stats[:, 4:5], scr_n[:], axis=mybir.AxisListType.X)
            scr_n2 = scr.tile([K, SW], F32, tag="scr_n")
            nc.vector.tensor_mul(scr_n2[:], x_v[:, 2, :], x_v[:, 2, :])
            nc.vector.reduce_sum(stats[:, 5:6], scr_n2[:], axis=mybir.AxisListType.X)

            # Transpose a_de halves once for ALL samples:
            # a_deT_*[tau, p] = a_de[p, tau (+128)] -> sample p//32, col block p%32
            a_te = psumt_pool.tile([K, K], BF16, tag="a_te")
            nc.tensor.transpose(a_te[:], a_de[:, 0:K], ident[:])
            a_to = psumt_pool.tile([K, K], BF16, tag="a_to")
            nc.tensor.transpose(a_to[:], a_de[:, K : 2 * K], ident[:])

            # Two zeroed DRAM scratch buffers for b_pad (zero regions written once)
            b_pads = [dram_pool.tile([BP_LEN], BF16, tag=f"bp{j}", name=f"b_pad{j}") for j in range(NS)]
            for bp in b_pads:
                nc.sync.dma_start(
                    out=bp[:].rearrange("(p g) -> p g", p=K), in_=zeros_bf[:]
                )

            for n in range(NS):
                # A_cols: [64 zero | a bf16 | 64 zero]; even/odd g columns come
                # from the two transpose halves.  a_odd = A_cols shifted one
                # column so every matmul weight slice is 4-byte aligned.
                a_cols = acols_pool.tile([K, A_W], BF16, tag="a_cols")
                nc.vector.memset(a_cols[:], 0.0)
                av = a_cols[:].rearrange("p (g two) -> p two g", two=2)
                nc.vector.tensor_copy(out=av[:, 0, 32:64], in_=a_te[:, 32 * n : 32 * n + 32])
                nc.vector.tensor_copy(out=av[:, 1, 32:64], in_=a_to[:, 32 * n : 32 * n + 32])
                a_odd = acols_pool.tile([K, A_W], BF16, tag="a_odd")
                nc.vector.tensor_copy(out=a_odd[:, 0 : A_W - 1], in_=a_cols[:, 1:A_W])

                # b_pad[128 + 256*p' + u] = b_de[32n+p', u]: contiguous writes
                bp = b_pads[n]
                nc.sync.dma_start(
                    out=bp[K : K + L].rearrange("(p u) -> p u", p=32),
                    in_=b_de[32 * n : 32 * n + 32, :],
                )

                # B_sh[tau, x] = b_pad[tau + x + 1]: one overlapping-read DMA
                b_sh = bsh_pool.tile([K, BW], BF16)
                bpa = bp[:]
                half = BW // 2
                for h in range(2):
                    src = bass.AP(
                        tensor=bpa.tensor,
                        offset=bpa.offset + 1 + h * half,
                        ap=[[1, K], [1, half]],
                    )
                    nc.gpsimd.dma_start(out=b_sh[:, h * half : (h + 1) * half], in_=src)

                # 65 accumulating matmuls; psum holds every corr lag once
                psum = psum_pool.tile([K, K], F32)
                for i in range(NT):
                    lhsT = (
                        a_cols[:, i : i + K]
                        if i % 2 == 0
                        else a_odd[:, i - 1 : i - 1 + K]
                    )
                    nc.tensor.matmul(
                        psum[:],
                        lhsT,
                        b_sh[:, K * i : K * i + K],
                        start=(i == 0),
                        stop=(i == NT - 1),
                    )

                # sum(c^2) -> stats col n (square on ScalarE, reduce on DVE)
                scr_c2 = scr.tile([K, K], F32, tag="scr_c2")
                nc.scalar.activation(
                    out=scr_c2[:], in_=psum[:],
                    func=mybir.ActivationFunctionType.Square,
                )
                nc.vector.reduce_sum(
                    stats[:, n : n + 1], scr_c2[:], axis=mybir.AxisListType.X
                )

        if "bce" in parts:
            # ---- BCE: relu(x) - x*t + ln(1 + exp(-|x|)), per-class sums ----
            t_sb = bce_pool.tile([K, FW], F32)
            nc.sync.dma_start(
                out=t_sb[:],
                in_=targ.rearrange("n l c -> (n l c)").rearrange("(p f) -> p f", p=K),
            )
            ax = bce_pool.tile([K, FW], F32)
            nc.scalar.activation(ax[:], x_sb[:], mybir.ActivationFunctionType.Abs)
            ex = bce_pool.tile([K, FW], F32)
            nc.scalar.activation(
                ex[:], ax[:], mybir.ActivationFunctionType.Exp, scale=-1.0
            )
            sp = bce_pool.tile([K, FW], F32)
            nc.scalar.activation(
                sp[:], ex[:], mybir.ActivationFunctionType.Ln, bias=1.0
            )
            rx = bce_pool.tile([K, FW], F32)
            nc.vector.tensor_scalar_max(rx[:], x_sb[:], 0.0)
            xt = bce_pool.tile([K, FW], F32)
            nc.vector.tensor_mul(xt[:], x_sb[:], t_sb[:])
            v = bce_pool.tile([K, FW], F32)
            nc.vector.tensor_sub(v[:], rx[:], xt[:])
            nc.vector.tensor_add(v[:], v[:], sp[:])
            v_view = v[:].rearrange("p (t c) -> p c t", c=C)
            nc.vector.reduce_sum(
                stats[:, 6 : 6 + C], v_view, axis=mybir.AxisListType.X
            )

        nc.sync.dma_start(out=out[:], in_=stats[:])


def _build(parts=FULL_PARTS):
    global _CACHED_NC
    if _CACHED_NC is not None and _CACHED_NC[0] == parts:
        return _CACHED_NC[1]
    nc = bacc.Bacc(
        "TRN2",
        target_bir_lowering=False,
        debug=False,
        enable_asserts=False,
        num_devices=N_CORES,
    )
    with tile.TileContext(nc) as tc:
        _kernel_body(tc, parts)
    nc.compile()
    _CACHED_NC = (parts, nc)
    return nc


def host_reduce(stats_list, weight):
    """Final scalar reduction over per-core [128, 16] stats, in float64."""
    w = np.asarray(weight, dtype=np.float64)
    bce_sum = 0.0
    prox = 0.0
    for stats in stats_list:
        s = np.asarray(stats, dtype=np.float64)
        ss = s[:, 0:4].sum(axis=0)
        sa = s[:, 4].reshape(NS, 32).sum(axis=1)
        sb = s[:, 5].reshape(NS, 32).sum(axis=1)
        prox += float((ss / np.sqrt(sa * sb)).sum())
        bce_sum += float((s[:, 6:9].sum(axis=0) * w).sum())
    loss = LAMBDA1 * bce_sum / (N_FULL * L * C) + LAMBDA2 * prox
    return np.float32(loss)


def kernel(predictions, targets, weight, trace=False):
    global LAST_RESULT
    predictions = np.ascontiguousarray(np.asarray(predictions, dtype=np.float32))
    targets = np.ascontiguousarray(np.asarray(targets, dtype=np.float32))
    weight = np.asarray(weight, dtype=np.float32)
    assert predictions.shape == (N_FULL, L, C), predictions.shape

    nc = _build()
    in_maps = [
        {
            "predictions": np.ascontiguousarray(predictions[k * NS : (k + 1) * NS]),
            "targets": np.ascontiguousarray(targets[k * NS : (k + 1) * NS]),
        }
        for k in range(N_CORES)
    ]
    LAST_RESULT = run_bass_kernel_spmd(
        nc, in_maps, core_ids=list(range(N_CORES)), trace=trace
    )
    stats_list = [r["out"] for r in LAST_RESULT.results]
    return host_reduce(stats_list, weight)


# revision 24
# speedup vs baseline: 2.5517x; 1.0579x over previous
"""Distributed Trainium2 kernel for BCESleepLoss.

loss = mean(weight_c * (softplus(x) - x*t)) + 1e-4 * sum_n sum_j corr_n[j]^2 / norm_n

where corr_n = full cross-correlation of predictions[n,:,1] with predictions[n,:,2]
and norm_n = sqrt(sum(s1^2) * sum(s2^2)).

Sharding: data-parallel over the batch dim N=32 -> 4 samples on each of 8 cores.
Each core emits per-partition partial stats [128, 16]; the host does the final
(tiny) reduction in float64.

Cross-correlation as matmuls: for each sample, with K=128,
  out[m', nu] += A_cols[:, i:i+128].T @ B_sh[:, 128*i : 128*i+128],  i = 0..64
where A_cols[tau, g] = a_pad[128*g + tau] (zero-padded reshape of s1, built
on-chip via PE transposes) and B_sh[tau, x] = b_pad[tau + x + 1] (128 shifted
copies of zero-padded s2, staged through a DRAM scratch so a single
overlapping-read DMA can build it).  The 128x128 PSUM tile then holds every
correlation lag exactly once (scrambled), so sum(out^2) == sum(corr^2).
Verified against np.convolve in float64.

All DRAM traffic is contiguous or chunky (the fine-grained stride-3 gathers
are de-strided on-chip); fused/accum InstISA ops are avoided (they fail at
runtime in this environment).
"""

import numpy as np

import concourse.bass as bass
import concourse.mybir as mybir
import concourse.tile as tile
from concourse import bacc
from concourse.bass_utils import run_bass_kernel_spmd
from concourse.masks import make_identity

# Problem constants (hardcoded; kernel.py must be self-contained).
N_FULL = 32
L = 8192
C = 3
LAMBDA1 = 1.0
LAMBDA2 = 1e-4

N_CORES = 8
NS = N_FULL // N_CORES  # samples per core = 4

K = 128  # partition / tile size
G = L // K  # 64 columns of signal data per sample
NT = G + 1  # 65 accumulating matmuls per sample
A_W = 3 * G  # 192: A_cols width (64 zero | 64 data | 64 zero)
BP_LEN = 8576  # b_pad length = 128*67 (zeros | 8192 data | zeros)
BW = 8328  # B_sh width (matmuls read cols [0, 8320))
TPS = L // K  # 64: t-steps per partition in the de-strided [128, 256] layout

F32 = mybir.dt.float32
BF16 = mybir.dt.bfloat16

LAST_RESULT = None  # BassKernelResults of the most recent run (for test.py)
_CACHED_NC = None

FULL_PARTS = ("corr", "bce")


def _kernel_body(tc, parts=FULL_PARTS):
    nc = tc.nc
    pred = nc.dram_tensor("predictions", [NS, L, C], F32, kind="ExternalInput").ap()
    targ = nc.dram_tensor("targets", [NS, L, C], F32, kind="ExternalInput").ap()
    out = nc.dram_tensor("out", [K, 16], F32, kind="ExternalOutput").ap()

    FW = NS * L * C // K  # 768 cols in the flat [128, 768] input layout
    SW = NS * L // K  # 256 cols per de-strided signal

    with (
        tc.tile_pool(name="singles", bufs=1) as singles,
        tc.tile_pool(name="acols", bufs=2) as acols_pool,
        tc.tile_pool(name="bsh", bufs=3) as bsh_pool,
        tc.tile_pool(name="scr", bufs=2) as scr,
        tc.tile_pool(name="bce", bufs=1) as bce_pool,
        tc.tile_pool(name="psum", bufs=2, space="PSUM") as psum_pool,
        tc.tile_pool(name="psumt", bufs=1, space="PSUM") as psumt_pool,
        tc.tile_pool(name="dram", bufs=1, space="DRAM") as dram_pool,
    ):
        # Per-partition partial stats, one DMA out at the end.
        # cols 0:4 = sum(c^2) per sample; col 4 = sum(s1^2), col 5 = sum(s2^2)
        # (per-partition, sample = p // 32); cols 6:9 = per-class BCE sums.
        stats = singles.tile([K, 16], F32)
        nc.vector.memset(stats[:], 0.0)

        b_pads = []
        if "corr" in parts:
            zeros_bf = singles.tile([K, BP_LEN // K], BF16)
            nc.vector.memset(zeros_bf[:], 0.0)
            # Zero the b_pad scratch buffers first thing (gpsimd queue) so the
            # first sample's staging chain starts as early as possible.
            b_pads = [
                dram_pool.tile([BP_LEN], BF16, tag=f"bp{j}", name=f"b_pad{j}")
                for j in range(NS)
            ]
            for bp in b_pads:
                nc.gpsimd.dma_start(
                    out=bp[:].rearrange("(p g) -> p g", p=K), in_=zeros_bf[:]
                )

        # Contiguous input loads, shared by both loss terms.
        # x_sb[p, f] = pred_flat[768*p + f]; partition p holds sample p // 32.
        x_sb = bce_pool.tile([K, FW], F32)
        nc.sync.dma_start(
            out=x_sb[:],
            in_=pred.rearrange("n l c -> (n l c)").rearrange("(p f) -> p f", p=K),
        )
        x_v = x_sb[:].rearrange("p (t c) -> p c t", c=C)

        if "corr" in parts:
            ident = singles.tile([K, K], BF16)
            make_identity(nc, ident[:])

            # De-stride s1/s2 (stride-3 SBUF reads on DVE) + cast to bf16:
            # a_de[p, u] = s1[p//32][256*(p%32) + u]
            a_de = singles.tile([K, SW], BF16)
            nc.vector.tensor_copy(out=a_de[:], in_=x_v[:, 1, :])
            b_de = singles.tile([K, SW], BF16)
            nc.vector.tensor_copy(out=b_de[:], in_=x_v[:, 2, :])

            # norms in f32 from x_sb: per-partition partials (sample = p//32)
            scr_n = scr.tile([K, SW], F32, tag="scr_n")
            nc.vector.tensor_mul(scr_n[:], x_v[:, 1, :], x_v[:, 1, :])
            nc.vector.reduce_sum(stats[:, 4:5], scr_n[:], axis=mybir.AxisListType.X)
            scr_n2 = scr.tile([K, SW], F32, tag="scr_n")
            nc.vector.tensor_mul(scr_n2[:], x_v[:, 2, :], x_v[:, 2, :])
            nc.vector.reduce_sum(stats[:, 5:6], scr_n2[:], axis=mybir.AxisListType.X)

            # Transpose a_de halves once for ALL samples:
            # a_deT_*[tau, p] = a_de[p, tau (+128)] -> sample p//32, col block p%32
            a_te = psumt_pool.tile([K, K], BF16, tag="a_te")
            nc.tensor.transpose(a_te[:], a_de[:, 0:K], ident[:])
            a_to = psumt_pool.tile([K, K], BF16, tag="a_to")
            nc.tensor.transpose(a_to[:], a_de[:, K : 2 * K], ident[:])

            for n in range(NS):
                # A_cols: [64 zero | a bf16 | 64 zero]; even/odd g columns come
                # from the two transpose halves.  a_odd = A_cols shifted one
                # column so every matmul weight slice is 4-byte aligned.
                a_cols = acols_pool.tile([K, A_W], BF16, tag="a_cols")
                nc.vector.memset(a_cols[:], 0.0)
                av = a_cols[:].rearrange("p (g two) -> p two g", two=2)
                nc.vector.tensor_copy(out=av[:, 0, 32:64], in_=a_te[:, 32 * n : 32 * n + 32])
                nc.vector.tensor_copy(out=av[:, 1, 32:64], in_=a_to[:, 32 * n : 32 * n + 32])
                a_odd = acols_pool.tile([K, A_W], BF16, tag="a_odd")
                nc.vector.tensor_copy(out=a_odd[:, 0 : A_W - 1], in_=a_cols[:, 1:A_W])

                # b_pad[128 + 256*p' + u] = b_de[32n+p', u]: contiguous writes
                bp = b_pads[n]
                nc.sync.dma_start(
                    out=bp[K : K + L].rearrange("(p u) -> p u", p=32),
                    in_=b_de[32 * n : 32 * n + 32, :],
                )

                # B_sh[tau, x] = b_pad[tau + x + 1]: one overlapping-read DMA
                b_sh = bsh_pool.tile([K, BW], BF16)
                bpa = bp[:]
                qw = BW // 4
                for h in range(4):
                    src = bass.AP(
                        tensor=bpa.tensor,
                        offset=bpa.offset + 1 + h * qw,
                        ap=[[1, K], [1, qw]],
                    )
                    nc.gpsimd.dma_start(out=b_sh[:, h * qw : (h + 1) * qw], in_=src)

                # 65 accumulating matmuls; psum holds every corr lag once
                psum = psum_pool.tile([K, K], F32)
                for i in range(NT):
                    lhsT = (
                        a_cols[:, i : i + K]
                        if i % 2 == 0
                        else a_odd[:, i - 1 : i - 1 + K]
                    )
                    nc.tensor.matmul(
                        psum[:],
                        lhsT,
                        b_sh[:, K * i : K * i + K],
                        start=(i == 0),
                        stop=(i == NT - 1),
                    )

                # sum(c^2) -> stats col n (square on ScalarE, reduce on DVE)
                scr_c2 = scr.tile([K, K], F32, tag="scr_c2")
                nc.scalar.activation(
                    out=scr_c2[:], in_=psum[:],
                    func=mybir.ActivationFunctionType.Square,
                )
                nc.vector.reduce_sum(
                    stats[:, n : n + 1], scr_c2[:], axis=mybir.AxisListType.X
                )

        if "bce" in parts:
            # ---- BCE: relu(x) - x*t + ln(1 + exp(-|x|)), per-class sums ----
            t_sb = bce_pool.tile([K, FW], F32)
            nc.sync.dma_start(
                out=t_sb[:],
                in_=targ.rearrange("n l c -> (n l c)").rearrange("(p f) -> p f", p=K),
            )
            ax = bce_pool.tile([K, FW], F32)
            nc.scalar.activation(ax[:], x_sb[:], mybir.ActivationFunctionType.Abs)
            ex = bce_pool.tile([K, FW], F32)
            nc.scalar.activation(
                ex[:], ax[:], mybir.ActivationFunctionType.Exp, scale=-1.0
            )
            sp = bce_pool.tile([K, FW], F32)
            nc.scalar.activation(
                sp[:], ex[:], mybir.ActivationFunctionType.Ln, bias=1.0
            )
            rx = bce_pool.tile([K, FW], F32)
            nc.vector.tensor_scalar_max(rx[:], x_sb[:], 0.0)
            xt = bce_pool.tile([K, FW], F32)
            nc.vector.tensor_mul(xt[:], x_sb[:], t_sb[:])
            v = bce_pool.tile([K, FW], F32)
            nc.vector.tensor_sub(v[:], rx[:], xt[:])
            nc.vector.tensor_add(v[:], v[:], sp[:])
            v_view = v[:].rearrange("p (t c) -> p c t", c=C)
            nc.vector.reduce_sum(
                stats[:, 6 : 6 + C], v_view, axis=mybir.AxisListType.X
            )

        nc.sync.dma_start(out=out[:], in_=stats[:])


def _build(parts=FULL_PARTS):
    global _CACHED_NC
    if _CACHED_NC is not None and _CACHED_NC[0] == parts:
        return _CACHED_NC[1]
    nc = bacc.Bacc(
        "TRN2",
        target_bir_lowering=False,
        debug=False,
        enable_asserts=False,
        num_devices=N_CORES,
    )
    with tile.TileContext(nc) as tc:
        _kernel_body(tc, parts)
    nc.compile()
    _CACHED_NC = (parts, nc)
    return nc


def host_reduce(stats_list, weight):
    """Final scalar reduction over per-core [128, 16] stats, in float64."""
    w = np.asarray(weight, dtype=np.float64)
    bce_sum = 0.0
    prox = 0.0
    for stats in stats_list:
        s = np.asarray(stats, dtype=np.float64)
        ss = s[:, 0:4].sum(axis=0)
        sa = s[:, 4].reshape(NS, 32).sum(axis=1)
        sb = s[:, 5].reshape(NS, 32).sum(axis=1)
        prox += float((ss / np.sqrt(sa * sb)).sum())
        bce_sum += float((s[:, 6:9].sum(axis=0) * w).sum())
    loss = LAMBDA1 * bce_sum / (N_FULL * L * C) + LAMBDA2 * prox
    return np.float32(loss)


def kernel(predictions, targets, weight, trace=False):
    global LAST_RESULT
    predictions = np.ascontiguousarray(np.asarray(predictions, dtype=np.float32))
    targets = np.ascontiguousarray(np.asarray(targets, dtype=np.float32))
    weight = np.asarray(weight, dtype=np.float32)
    assert predictions.shape == (N_FULL, L, C), predictions.shape

    nc = _build()
    in_maps = [
        {
            "predictions": np.ascontiguousarray(predictions[k * NS : (k + 1) * NS]),
            "targets": np.ascontiguousarray(targets[k * NS : (k + 1) * NS]),
        }
        for k in range(N_CORES)
    ]
    LAST_RESULT = run_bass_kernel_spmd(
        nc, in_maps, core_ids=list(range(N_CORES)), trace=trace
    )
    stats_list = [r["out"] for r in LAST_RESULT.results]
    return host_reduce(stats_list, weight)


# revision 26
# speedup vs baseline: 2.7441x; 1.0754x over previous
"""Distributed Trainium2 kernel for BCESleepLoss.

loss = mean(weight_c * (softplus(x) - x*t)) + 1e-4 * sum_n sum_j corr_n[j]^2 / norm_n

where corr_n = full cross-correlation of predictions[n,:,1] with predictions[n,:,2]
and norm_n = sqrt(sum(s1^2) * sum(s2^2)).

Sharding: data-parallel over the batch dim N=32 -> 4 samples on each of 8 cores.
Each core emits per-partition partial stats [128, 16]; the host does the final
(tiny) reduction in float64.

Cross-correlation as matmuls: for each sample, with K=128,
  out[m', nu] += A_cols[:, i:i+128].T @ B_sh[:, 128*i : 128*i+128],  i = 0..64
where A_cols[tau, g] = a_pad[128*g + tau] (zero-padded reshape of s1, built
on-chip via PE transposes) and B_sh[tau, x] = b_pad[tau + x + 1] (128 shifted
copies of zero-padded s2, staged through a DRAM scratch so a single
overlapping-read DMA can build it).  The 128x128 PSUM tile then holds every
correlation lag exactly once (scrambled), so sum(out^2) == sum(corr^2).
Verified against np.convolve in float64.

All DRAM traffic is contiguous or chunky (the fine-grained stride-3 gathers
are de-strided on-chip); fused/accum InstISA ops are avoided (they fail at
runtime in this environment).
"""

import numpy as np

import concourse.bass as bass
import concourse.mybir as mybir
import concourse.tile as tile
from concourse import bacc
from concourse.bass_utils import run_bass_kernel_spmd
from concourse.masks import make_identity

# Problem constants (hardcoded; kernel.py must be self-contained).
N_FULL = 32
L = 8192
C = 3
LAMBDA1 = 1.0
LAMBDA2 = 1e-4

N_CORES = 8
NS = N_FULL // N_CORES  # samples per core = 4

K = 128  # partition / tile size
G = L // K  # 64 columns of signal data per sample
NT = G + 1  # 65 accumulating matmuls per sample
A_W = 3 * G  # 192: A_cols width (64 zero | 64 data | 64 zero)
BP_LEN = 8576  # b_pad length = 128*67 (zeros | 8192 data | zeros)
BW = 8328  # B_sh width (matmuls read cols [0, 8320))
TPS = L // K  # 64: t-steps per partition in the de-strided [128, 256] layout

F32 = mybir.dt.float32
BF16 = mybir.dt.bfloat16
FP8 = mybir.dt.float8e4  # e4m3: staging/matmul dtype (rel-err gate is 2e-2)

LAST_RESULT = None  # BassKernelResults of the most recent run (for test.py)
_CACHED_NC = None

FULL_PARTS = ("corr", "bce")


def _kernel_body(tc, parts=FULL_PARTS):
    nc = tc.nc
    pred = nc.dram_tensor("predictions", [NS, L, C], F32, kind="ExternalInput").ap()
    targ = nc.dram_tensor("targets", [NS, L, C], F32, kind="ExternalInput").ap()
    out = nc.dram_tensor("out", [K, 16], F32, kind="ExternalOutput").ap()

    FW = NS * L * C // K  # 768 cols in the flat [128, 768] input layout
    SW = NS * L // K  # 256 cols per de-strided signal

    with (
        tc.tile_pool(name="singles", bufs=1) as singles,
        tc.tile_pool(name="acols", bufs=2) as acols_pool,
        tc.tile_pool(name="bsh", bufs=3) as bsh_pool,
        tc.tile_pool(name="scr", bufs=2) as scr,
        tc.tile_pool(name="bce", bufs=1) as bce_pool,
        tc.tile_pool(name="psum", bufs=2, space="PSUM") as psum_pool,
        tc.tile_pool(name="psumt", bufs=1, space="PSUM") as psumt_pool,
        tc.tile_pool(name="dram", bufs=1, space="DRAM") as dram_pool,
    ):
        # Per-partition partial stats, one DMA out at the end.
        # cols 0:4 = sum(c^2) per sample; col 4 = sum(s1^2), col 5 = sum(s2^2)
        # (per-partition, sample = p // 32); cols 6:9 = per-class BCE sums.
        stats = singles.tile([K, 16], F32)
        nc.vector.memset(stats[:], 0.0)

        b_pads = []
        if "corr" in parts:
            zeros_bf = singles.tile([K, NS * BP_LEN // K], FP8)
            nc.vector.memset(zeros_bf[:], 0.0)
            # One zeroed DRAM scratch holding all four b_pads; zero-filled by a
            # single DMA first thing so sample 0's staging starts ASAP.
            b_pad_all = dram_pool.tile([NS * BP_LEN], FP8, name="b_pad_all")
            nc.gpsimd.dma_start(
                out=b_pad_all[:].rearrange("(p g) -> p g", p=K), in_=zeros_bf[:]
            )
            b_pads = [b_pad_all[j * BP_LEN : (j + 1) * BP_LEN] for j in range(NS)]

        # Contiguous input loads, shared by both loss terms.
        # x_sb[p, f] = pred_flat[768*p + f]; partition p holds sample p // 32.
        x_sb = bce_pool.tile([K, FW], F32)
        nc.sync.dma_start(
            out=x_sb[:],
            in_=pred.rearrange("n l c -> (n l c)").rearrange("(p f) -> p f", p=K),
        )
        x_v = x_sb[:].rearrange("p (t c) -> p c t", c=C)

        if "corr" in parts:
            ident = singles.tile([K, K], BF16)
            make_identity(nc, ident[:])

            # De-stride s1/s2 (stride-3 SBUF reads on DVE) + cast to bf16:
            # a_de[p, u] = s1[p//32][256*(p%32) + u]
            a_de = singles.tile([K, SW], BF16)
            nc.vector.tensor_copy(out=a_de[:], in_=x_v[:, 1, :])
            b_de = singles.tile([K, SW], FP8)
            nc.vector.tensor_copy(out=b_de[:], in_=x_v[:, 2, :])

            # norms in f32 from x_sb: per-partition partials (sample = p//32)
            scr_n = scr.tile([K, SW], F32, tag="scr_n")
            nc.vector.tensor_mul(scr_n[:], x_v[:, 1, :], x_v[:, 1, :])
            nc.vector.reduce_sum(stats[:, 4:5], scr_n[:], axis=mybir.AxisListType.X)
            scr_n2 = scr.tile([K, SW], F32, tag="scr_n")
            nc.vector.tensor_mul(scr_n2[:], x_v[:, 2, :], x_v[:, 2, :])
            nc.vector.reduce_sum(stats[:, 5:6], scr_n2[:], axis=mybir.AxisListType.X)

            # Transpose a_de halves once for ALL samples:
            # a_deT_*[tau, p] = a_de[p, tau (+128)] -> sample p//32, col block p%32
            a_te = psumt_pool.tile([K, K], BF16, tag="a_te")
            nc.tensor.transpose(a_te[:], a_de[:, 0:K], ident[:])
            a_to = psumt_pool.tile([K, K], BF16, tag="a_to")
            nc.tensor.transpose(a_to[:], a_de[:, K : 2 * K], ident[:])

            for n in range(NS):
                # A_cols: [64 zero | a bf16 | 64 zero]; even/odd g columns come
                # from the two transpose halves.  a_odd = A_cols shifted one
                # column so every matmul weight slice is 4-byte aligned.
                a_cols = acols_pool.tile([K, A_W], FP8, tag="a_cols")
                nc.vector.memset(a_cols[:], 0.0)
                av = a_cols[:].rearrange("p (g two) -> p two g", two=2)
                nc.vector.tensor_copy(out=av[:, 0, 32:64], in_=a_te[:, 32 * n : 32 * n + 32])
                nc.vector.tensor_copy(out=av[:, 1, 32:64], in_=a_to[:, 32 * n : 32 * n + 32])
                # 3 column-shifted copies so every weight slice is 4B-aligned
                a_phs = [a_cols]
                for r in range(1, 4):
                    a_ph = acols_pool.tile([K, A_W], FP8, tag=f"a_ph{r}", name=f"a_ph{r}")
                    nc.vector.tensor_copy(out=a_ph[:, 0 : A_W - r], in_=a_cols[:, r:A_W])
                    a_phs.append(a_ph)

                # b_pad[128 + 256*p' + u] = b_de[32n+p', u]: contiguous writes
                bp = b_pads[n]
                nc.sync.dma_start(
                    out=bp[K : K + L].rearrange("(p u) -> p u", p=32),
                    in_=b_de[32 * n : 32 * n + 32, :],
                )

                # B_sh[tau, x] = b_pad[tau + x + 1]: one overlapping-read DMA
                b_sh = bsh_pool.tile([K, BW], FP8)
                bpa = bp[:]
                qw = BW // 4
                for h in range(4):
                    src = bass.AP(
                        tensor=bpa.tensor,
                        offset=bpa.offset + 1 + h * qw,
                        ap=[[1, K], [1, qw]],
                    )
                    nc.gpsimd.dma_start(out=b_sh[:, h * qw : (h + 1) * qw], in_=src)

                # 65 accumulating matmuls; psum holds every corr lag once
                psum = psum_pool.tile([K, K], F32)
                for i in range(NT):
                    r = i % 4
                    lhsT = a_phs[r][:, i - r : i - r + K]
                    nc.tensor.matmul(
                        psum[:],
                        lhsT,
                        b_sh[:, K * i : K * i + K],
                        start=(i == 0),
                        stop=(i == NT - 1),
                    )

                # sum(c^2) -> stats col n (square on ScalarE, reduce on DVE)
                scr_c2 = scr.tile([K, K], F32, tag="scr_c2")
                nc.scalar.activation(
                    out=scr_c2[:], in_=psum[:],
                    func=mybir.ActivationFunctionType.Square,
                )
                nc.vector.reduce_sum(
                    stats[:, n : n + 1], scr_c2[:], axis=mybir.AxisListType.X
                )

        if "bce" in parts:
            # ---- BCE: relu(x) - x*t + ln(1 + exp(-|x|)), per-class sums ----
            t_sb = bce_pool.tile([K, FW], F32)
            nc.sync.dma_start(
                out=t_sb[:],
                in_=targ.rearrange("n l c -> (n l c)").rearrange("(p f) -> p f", p=K),
            )
            ax = bce_pool.tile([K, FW], F32)
            nc.scalar.activation(ax[:], x_sb[:], mybir.ActivationFunctionType.Abs)
            ex = bce_pool.tile([K, FW], F32)
            nc.scalar.activation(
                ex[:], ax[:], mybir.ActivationFunctionType.Exp, scale=-1.0
            )
            sp = bce_pool.tile([K, FW], F32)
            nc.scalar.activation(
                sp[:], ex[:], mybir.ActivationFunctionType.Ln, bias=1.0
            )
            rx = bce_pool.tile([K, FW], F32)
            nc.vector.tensor_scalar_max(rx[:], x_sb[:], 0.0)
            xt = bce_pool.tile([K, FW], F32)
            nc.vector.tensor_mul(xt[:], x_sb[:], t_sb[:])
            v = bce_pool.tile([K, FW], F32)
            nc.vector.tensor_sub(v[:], rx[:], xt[:])
            nc.vector.tensor_add(v[:], v[:], sp[:])
            v_view = v[:].rearrange("p (t c) -> p c t", c=C)
            nc.vector.reduce_sum(
                stats[:, 6 : 6 + C], v_view, axis=mybir.AxisListType.X
            )

        nc.sync.dma_start(out=out[:], in_=stats[:])


def _build(parts=FULL_PARTS):
    global _CACHED_NC
    if _CACHED_NC is not None and _CACHED_NC[0] == parts:
        return _CACHED_NC[1]
    nc = bacc.Bacc(
        "TRN2",
        target_bir_lowering=False,
        debug=False,
        enable_asserts=False,
        num_devices=N_CORES,
    )
    with tile.TileContext(nc) as tc:
        _kernel_body(tc, parts)
    nc.compile()
    _CACHED_NC = (parts, nc)
    return nc


def host_reduce(stats_list, weight):
    """Final scalar reduction over per-core [128, 16] stats, in float64."""
    w = np.asarray(weight, dtype=np.float64)
    bce_sum = 0.0
    prox = 0.0
    for stats in stats_list:
        s = np.asarray(stats, dtype=np.float64)
        ss = s[:, 0:4].sum(axis=0)
        sa = s[:, 4].reshape(NS, 32).sum(axis=1)
        sb = s[:, 5].reshape(NS, 32).sum(axis=1)
        prox += float((ss / np.sqrt(sa * sb)).sum())
        bce_sum += float((s[:, 6:9].sum(axis=0) * w).sum())
    loss = LAMBDA1 * bce_sum / (N_FULL * L * C) + LAMBDA2 * prox
    return np.float32(loss)


def kernel(predictions, targets, weight, trace=False):
    global LAST_RESULT
    predictions = np.ascontiguousarray(np.asarray(predictions, dtype=np.float32))
    targets = np.ascontiguousarray(np.asarray(targets, dtype=np.float32))
    weight = np.asarray(weight, dtype=np.float32)
    assert predictions.shape == (N_FULL, L, C), predictions.shape

    nc = _build()
    in_maps = [
        {
            "predictions": np.ascontiguousarray(predictions[k * NS : (k + 1) * NS]),
            "targets": np.ascontiguousarray(targets[k * NS : (k + 1) * NS]),
        }
        for k in range(N_CORES)
    ]
    LAST_RESULT = run_bass_kernel_spmd(
        nc, in_maps, core_ids=list(range(N_CORES)), trace=trace
    )
    stats_list = [r["out"] for r in LAST_RESULT.results]
    return host_reduce(stats_list, weight)


# revision 28
# speedup vs baseline: 3.0876x; 1.1252x over previous
"""Distributed Trainium2 kernel for BCESleepLoss.

loss = mean(weight_c * (softplus(x) - x*t)) + 1e-4 * sum_n sum_j corr_n[j]^2 / norm_n

where corr_n = full cross-correlation of predictions[n,:,1] with predictions[n,:,2]
and norm_n = sqrt(sum(s1^2) * sum(s2^2)).

Sharding: data-parallel over the batch dim N=32 -> 4 samples on each of 8 cores.
Each core emits per-partition partial stats [128, 16]; the host does the final
(tiny) reduction in float64.

Cross-correlation as matmuls: for each sample, with K=128,
  out[m', nu] += A_cols[:, i:i+128].T @ B_sh[:, 128*i : 128*i+128],  i = 0..64
where A_cols[tau, g] = a_pad[128*g + tau] (zero-padded reshape of s1, built
on-chip via PE transposes) and B_sh[tau, x] = b_pad[tau + x + 1] (128 shifted
copies of zero-padded s2, staged through a DRAM scratch so a single
overlapping-read DMA can build it).  The 128x128 PSUM tile then holds every
correlation lag exactly once (scrambled), so sum(out^2) == sum(corr^2).
Verified against np.convolve in float64.

All DRAM traffic is contiguous or chunky (the fine-grained stride-3 gathers
are de-strided on-chip); fused/accum InstISA ops are avoided (they fail at
runtime in this environment).
"""

import numpy as np

import concourse.bass as bass
import concourse.mybir as mybir
import concourse.tile as tile
from concourse import bacc
from concourse.bass_utils import run_bass_kernel_spmd
from concourse.masks import make_identity

# Problem constants (hardcoded; kernel.py must be self-contained).
N_FULL = 32
L = 8192
C = 3
LAMBDA1 = 1.0
LAMBDA2 = 1e-4

N_CORES = 8
NS = N_FULL // N_CORES  # samples per core = 4

K = 128  # partition / tile size
G = L // K  # 64 columns of signal data per sample
NT = G + 1  # 65 accumulating matmuls per sample
A_W = 3 * G  # 192: A_cols width (64 zero | 64 data | 64 zero)
BP_LEN = 8576  # b_pad length = 128*67 (zeros | 8192 data | zeros)
BW = 8328  # B_sh width (matmuls read cols [0, 8320))
TPS = L // K  # 64: t-steps per partition in the de-strided [128, 256] layout

F32 = mybir.dt.float32
BF16 = mybir.dt.bfloat16
FP8 = mybir.dt.float8e4  # e4m3: staging/matmul dtype (rel-err gate is 2e-2)

LAST_RESULT = None  # BassKernelResults of the most recent run (for test.py)
_CACHED_NC = None

FULL_PARTS = ("corr", "bce")


def _kernel_body(tc, parts=FULL_PARTS):
    nc = tc.nc
    pred = nc.dram_tensor("predictions", [NS, L, C], F32, kind="ExternalInput").ap()
    targ = nc.dram_tensor("targets", [NS, L, C], F32, kind="ExternalInput").ap()
    out = nc.dram_tensor("out", [K, 16], F32, kind="ExternalOutput").ap()

    FW = NS * L * C // K  # 768 cols in the flat [128, 768] input layout
    SW = NS * L // K  # 256 cols per de-strided signal

    with (
        tc.tile_pool(name="singles", bufs=1) as singles,
        tc.tile_pool(name="acols", bufs=2) as acols_pool,
        tc.tile_pool(name="bsh", bufs=4) as bsh_pool,
        tc.tile_pool(name="scr", bufs=2) as scr,
        tc.tile_pool(name="bce", bufs=1) as bce_pool,
        tc.tile_pool(name="psum", bufs=2, space="PSUM") as psum_pool,
        tc.tile_pool(name="psumt", bufs=1, space="PSUM") as psumt_pool,
        tc.tile_pool(name="dram", bufs=1, space="DRAM") as dram_pool,
    ):
        # Per-partition partial stats, one DMA out at the end.
        # cols 0:4 = sum(c^2) per sample; col 4 = sum(s1^2), col 5 = sum(s2^2)
        # (per-partition, sample = p // 32); cols 6:9 = per-class BCE sums.
        stats = singles.tile([K, 16], F32)
        nc.vector.memset(stats[:], 0.0)

        if "corr" in parts:
            zeros_bf = singles.tile([K, NS * BP_LEN // K], FP8)
            nc.vector.memset(zeros_bf[:], 0.0)
            # One zeroed DRAM scratch holding all four b_pads; zero-filled by a
            # single DMA first thing so sample 0's staging starts ASAP.
            b_pad_all = dram_pool.tile([NS * BP_LEN], FP8, name="b_pad_all")
            nc.gpsimd.dma_start(
                out=b_pad_all[:].rearrange("(p g) -> p g", p=K), in_=zeros_bf[:]
            )

        # Contiguous input loads, shared by both loss terms.
        # x_sb[p, f] = pred_flat[768*p + f]; partition p holds sample p // 32.
        x_sb = bce_pool.tile([K, FW], F32)
        nc.sync.dma_start(
            out=x_sb[:],
            in_=pred.rearrange("n l c -> (n l c)").rearrange("(p f) -> p f", p=K),
        )
        x_v = x_sb[:].rearrange("p (t c) -> p c t", c=C)

        if "corr" in parts:
            ident = singles.tile([K, K], BF16)
            make_identity(nc, ident[:])

            # De-stride s1/s2 (stride-3 SBUF reads on DVE) + cast to bf16:
            # a_de[p, u] = s1[p//32][256*(p%32) + u]
            a_de = singles.tile([K, SW], BF16)
            nc.vector.tensor_copy(out=a_de[:], in_=x_v[:, 1, :])
            b_de = singles.tile([K, SW], FP8)
            nc.vector.tensor_copy(out=b_de[:], in_=x_v[:, 2, :])

            # All four b_pad data regions in ONE DMA (contiguous 256B writes),
            # then the B_sh builds, emitted earliest so the matmul pipeline is
            # never starved: B_sh[tau, x] = b_pad[tau + x + 1].
            bpa = b_pad_all[:]
            for n in range(NS):
                nc.sync.dma_start(
                    out=bass.AP(
                        tensor=bpa.tensor, offset=bpa.offset + n * BP_LEN + K,
                        ap=[[SW, 32], [1, SW]],
                    ),
                    in_=b_de[32 * n : 32 * n + 32, :],
                )
            b_shs = []
            qw = BW // 4
            for n in range(NS):
                b_sh = bsh_pool.tile([K, BW], FP8, name=f"b_sh{n}")
                for h in range(4):
                    qsrc = bass.AP(
                        tensor=bpa.tensor,
                        offset=bpa.offset + n * BP_LEN + 1 + h * qw,
                        ap=[[1, K], [1, qw]],
                    )
                    nc.gpsimd.dma_start(out=b_sh[:, h * qw : (h + 1) * qw], in_=qsrc)
                b_shs.append(b_sh)

            # norms in f32 from x_sb: per-partition partials (sample = p//32)
            scr_n = scr.tile([K, SW], F32, tag="scr_n")
            nc.vector.tensor_mul(scr_n[:], x_v[:, 1, :], x_v[:, 1, :])
            nc.vector.reduce_sum(stats[:, 4:5], scr_n[:], axis=mybir.AxisListType.X)
            scr_n2 = scr.tile([K, SW], F32, tag="scr_n")
            nc.vector.tensor_mul(scr_n2[:], x_v[:, 2, :], x_v[:, 2, :])
            nc.vector.reduce_sum(stats[:, 5:6], scr_n2[:], axis=mybir.AxisListType.X)

            # Transpose a_de halves once for ALL samples:
            # a_deT_*[tau, p] = a_de[p, tau (+128)] -> sample p//32, col block p%32
            a_te = psumt_pool.tile([K, K], BF16, tag="a_te")
            nc.tensor.transpose(a_te[:], a_de[:, 0:K], ident[:])
            a_to = psumt_pool.tile([K, K], BF16, tag="a_to")
            nc.tensor.transpose(a_to[:], a_de[:, K : 2 * K], ident[:])

            for n in range(NS):
                # A_cols: [64 zero | a bf16 | 64 zero]; even/odd g columns come
                # from the two transpose halves.  a_odd = A_cols shifted one
                # column so every matmul weight slice is 4-byte aligned.
                a_cols = acols_pool.tile([K, A_W], FP8, tag="a_cols")
                nc.vector.memset(a_cols[:], 0.0)
                av = a_cols[:].rearrange("p (g two) -> p two g", two=2)
                nc.vector.tensor_copy(out=av[:, 0, 32:64], in_=a_te[:, 32 * n : 32 * n + 32])
                nc.vector.tensor_copy(out=av[:, 1, 32:64], in_=a_to[:, 32 * n : 32 * n + 32])
                # 3 column-shifted copies so every weight slice is 4B-aligned
                a_phs = [a_cols]
                for r in range(1, 4):
                    a_ph = acols_pool.tile([K, A_W], FP8, tag=f"a_ph{r}", name=f"a_ph{r}")
                    nc.vector.tensor_copy(out=a_ph[:, 0 : A_W - r], in_=a_cols[:, r:A_W])
                    a_phs.append(a_ph)

                b_sh = b_shs[n]

                # 65 accumulating matmuls; psum holds every corr lag once
                psum = psum_pool.tile([K, K], F32)
                for i in range(NT):
                    r = i % 4
                    lhsT = a_phs[r][:, i - r : i - r + K]
                    nc.tensor.matmul(
                        psum[:],
                        lhsT,
                        b_sh[:, K * i : K * i + K],
                        start=(i == 0),
                        stop=(i == NT - 1),
                    )

                # sum(c^2) -> stats col n (square on ScalarE, reduce on DVE)
                scr_c2 = scr.tile([K, K], F32, tag="scr_c2")
                nc.scalar.activation(
                    out=scr_c2[:], in_=psum[:],
                    func=mybir.ActivationFunctionType.Square,
                )
                nc.vector.reduce_sum(
                    stats[:, n : n + 1], scr_c2[:], axis=mybir.AxisListType.X
                )

        if "bce" in parts:
            # ---- BCE: relu(x) - x*t + ln(1 + exp(-|x|)), per-class sums ----
            t_sb = bce_pool.tile([K, FW], F32)
            nc.sync.dma_start(
                out=t_sb[:],
                in_=targ.rearrange("n l c -> (n l c)").rearrange("(p f) -> p f", p=K),
            )
            ax = bce_pool.tile([K, FW], F32)
            nc.scalar.activation(ax[:], x_sb[:], mybir.ActivationFunctionType.Abs)
            ex = bce_pool.tile([K, FW], F32)
            nc.scalar.activation(
                ex[:], ax[:], mybir.ActivationFunctionType.Exp, scale=-1.0
            )
            sp = bce_pool.tile([K, FW], F32)
            nc.scalar.activation(
                sp[:], ex[:], mybir.ActivationFunctionType.Ln, bias=1.0
            )
            rx = bce_pool.tile([K, FW], F32)
            nc.vector.tensor_scalar_max(rx[:], x_sb[:], 0.0)
            xt = bce_pool.tile([K, FW], F32)
            nc.vector.tensor_mul(xt[:], x_sb[:], t_sb[:])
            v = bce_pool.tile([K, FW], F32)
            nc.vector.tensor_sub(v[:], rx[:], xt[:])
            nc.vector.tensor_add(v[:], v[:], sp[:])
            v_view = v[:].rearrange("p (t c) -> p c t", c=C)
            nc.vector.reduce_sum(
                stats[:, 6 : 6 + C], v_view, axis=mybir.AxisListType.X
            )

        nc.sync.dma_start(out=out[:], in_=stats[:])


def _build(parts=FULL_PARTS):
    global _CACHED_NC
    if _CACHED_NC is not None and _CACHED_NC[0] == parts:
        return _CACHED_NC[1]
    nc = bacc.Bacc(
        "TRN2",
        target_bir_lowering=False,
        debug=False,
        enable_asserts=False,
        num_devices=N_CORES,
    )
    with tile.TileContext(nc) as tc:
        _kernel_body(tc, parts)
    nc.compile()
    _CACHED_NC = (parts, nc)
    return nc


def host_reduce(stats_list, weight):
    """Final scalar reduction over per-core [128, 16] stats, in float64."""
    w = np.asarray(weight, dtype=np.float64)
    bce_sum = 0.0
    prox = 0.0
    for stats in stats_list:
        s = np.asarray(stats, dtype=np.float64)
        ss = s[:, 0:4].sum(axis=0)
        sa = s[:, 4].reshape(NS, 32).sum(axis=1)
        sb = s[:, 5].reshape(NS, 32).sum(axis=1)
        prox += float((ss / np.sqrt(sa * sb)).sum())
        bce_sum += float((s[:, 6:9].sum(axis=0) * w).sum())
    loss = LAMBDA1 * bce_sum / (N_FULL * L * C) + LAMBDA2 * prox
    return np.float32(loss)


def kernel(predictions, targets, weight, trace=False):
    global LAST_RESULT
    predictions = np.ascontiguousarray(np.asarray(predictions, dtype=np.float32))
    targets = np.ascontiguousarray(np.asarray(targets, dtype=np.float32))
    weight = np.asarray(weight, dtype=np.float32)
    assert predictions.shape == (N_FULL, L, C), predictions.shape

    nc = _build()
    in_maps = [
        {
            "predictions": np.ascontiguousarray(predictions[k * NS : (k + 1) * NS]),
            "targets": np.ascontiguousarray(targets[k * NS : (k + 1) * NS]),
        }
        for k in range(N_CORES)
    ]
    LAST_RESULT = run_bass_kernel_spmd(
        nc, in_maps, core_ids=list(range(N_CORES)), trace=trace
    )
    stats_list = [r["out"] for r in LAST_RESULT.results]
    return host_reduce(stats_list, weight)


# revision 29
# speedup vs baseline: 3.1094x; 1.0071x over previous
"""Distributed Trainium2 kernel for BCESleepLoss.

loss = mean(weight_c * (softplus(x) - x*t)) + 1e-4 * sum_n sum_j corr_n[j]^2 / norm_n

where corr_n = full cross-correlation of predictions[n,:,1] with predictions[n,:,2]
and norm_n = sqrt(sum(s1^2) * sum(s2^2)).

Sharding: data-parallel over the batch dim N=32 -> 4 samples on each of 8 cores.
Each core emits per-partition partial stats [128, 16]; the host does the final
(tiny) reduction in float64.

Cross-correlation as matmuls: for each sample, with K=128,
  out[m', nu] += A_cols[:, i:i+128].T @ B_sh[:, 128*i : 128*i+128],  i = 0..64
where A_cols[tau, g] = a_pad[128*g + tau] (zero-padded reshape of s1, built
on-chip via PE transposes) and B_sh[tau, x] = b_pad[tau + x + 1] (128 shifted
copies of zero-padded s2, staged through a DRAM scratch so a single
overlapping-read DMA can build it).  The 128x128 PSUM tile then holds every
correlation lag exactly once (scrambled), so sum(out^2) == sum(corr^2).
Verified against np.convolve in float64.

All DRAM traffic is contiguous or chunky (the fine-grained stride-3 gathers
are de-strided on-chip); fused/accum InstISA ops are avoided (they fail at
runtime in this environment).
"""

import numpy as np

import concourse.bass as bass
import concourse.mybir as mybir
import concourse.tile as tile
from concourse import bacc
from concourse.bass_utils import run_bass_kernel_spmd
from concourse.masks import make_identity

# Problem constants (hardcoded; kernel.py must be self-contained).
N_FULL = 32
L = 8192
C = 3
LAMBDA1 = 1.0
LAMBDA2 = 1e-4

N_CORES = 8
NS = N_FULL // N_CORES  # samples per core = 4

K = 128  # partition / tile size
G = L // K  # 64 columns of signal data per sample
NT = G + 1  # 65 accumulating matmuls per sample
A_W = 3 * G  # 192: A_cols width (64 zero | 64 data | 64 zero)
BP_LEN = 8576  # b_pad length = 128*67 (zeros | 8192 data | zeros)
BW = 8328  # B_sh width (matmuls read cols [0, 8320))
TPS = L // K  # 64: t-steps per partition in the de-strided [128, 256] layout

F32 = mybir.dt.float32
BF16 = mybir.dt.bfloat16
FP8 = mybir.dt.float8e4  # e4m3: staging/matmul dtype (rel-err gate is 2e-2)

LAST_RESULT = None  # BassKernelResults of the most recent run (for test.py)
_CACHED_NC = None

FULL_PARTS = ("corr", "bce")


def _kernel_body(tc, parts=FULL_PARTS):
    nc = tc.nc
    pred = nc.dram_tensor("predictions", [NS, L, C], F32, kind="ExternalInput").ap()
    targ = nc.dram_tensor("targets", [NS, L, C], F32, kind="ExternalInput").ap()
    out = nc.dram_tensor("out", [K, 16], F32, kind="ExternalOutput").ap()

    FW = NS * L * C // K  # 768 cols in the flat [128, 768] input layout
    SW = NS * L // K  # 256 cols per de-strided signal

    with (
        tc.tile_pool(name="singles", bufs=1) as singles,
        tc.tile_pool(name="acols", bufs=2) as acols_pool,
        tc.tile_pool(name="bsh", bufs=4) as bsh_pool,
        tc.tile_pool(name="scr", bufs=2) as scr,
        tc.tile_pool(name="bce", bufs=1) as bce_pool,
        tc.tile_pool(name="psum", bufs=2, space="PSUM") as psum_pool,
        tc.tile_pool(name="psumt", bufs=1, space="PSUM") as psumt_pool,
        tc.tile_pool(name="dram", bufs=1, space="DRAM") as dram_pool,
    ):
        # Per-partition partial stats, one DMA out at the end.
        # cols 0:4 = sum(c^2) per sample; col 4 = sum(s1^2), col 5 = sum(s2^2)
        # (per-partition, sample = p // 32); cols 6:9 = per-class BCE sums.
        stats = singles.tile([K, 16], F32)
        nc.vector.memset(stats[:], 0.0)

        if "corr" in parts:
            zeros_bf = singles.tile([K, NS * BP_LEN // K], FP8)
            nc.vector.memset(zeros_bf[:], 0.0)
            # One zeroed DRAM scratch holding all four b_pads; zero-filled by a
            # single DMA first thing so sample 0's staging starts ASAP.
            b_pad_all = dram_pool.tile([NS * BP_LEN], FP8, name="b_pad_all")
            nc.gpsimd.dma_start(
                out=b_pad_all[:].rearrange("(p g) -> p g", p=K), in_=zeros_bf[:]
            )

        # Contiguous input loads, shared by both loss terms.
        # x_sb[p, f] = pred_flat[768*p + f]; partition p holds sample p // 32.
        x_sb = bce_pool.tile([K, FW], F32)
        nc.sync.dma_start(
            out=x_sb[:],
            in_=pred.rearrange("n l c -> (n l c)").rearrange("(p f) -> p f", p=K),
        )
        x_v = x_sb[:].rearrange("p (t c) -> p c t", c=C)

        if "corr" in parts:
            ident = singles.tile([K, K], BF16)
            make_identity(nc, ident[:])

            # De-stride s1/s2 (stride-3 SBUF reads on DVE) + cast to bf16:
            # a_de[p, u] = s1[p//32][256*(p%32) + u]
            b_de = singles.tile([K, SW], FP8)
            nc.vector.tensor_copy(out=b_de[:], in_=x_v[:, 2, :])

            # All four b_pad data regions in ONE DMA (contiguous 256B writes),
            # then the B_sh builds, emitted earliest so the matmul pipeline is
            # never starved: B_sh[tau, x] = b_pad[tau + x + 1].
            bpa = b_pad_all[:]
            for n in range(NS):
                nc.sync.dma_start(
                    out=bass.AP(
                        tensor=bpa.tensor, offset=bpa.offset + n * BP_LEN + K,
                        ap=[[SW, 32], [1, SW]],
                    ),
                    in_=b_de[32 * n : 32 * n + 32, :],
                )
            # B_sh in four SEPARATE chunk tiles with 128-aligned boundaries so
            # each matmul's dependency is exactly one chunk's DMA.
            CH_OFF = [0, 2048, 4096, 6144]
            CH_W = [2048, 2048, 2048, BW - 6144]
            b_shs = []
            for n in range(NS):
                chunks = []
                for h in range(4):
                    b_shc = bsh_pool.tile(
                        [K, CH_W[h]], FP8, tag=f"bshc{h}", name=f"b_sh{n}c{h}"
                    )
                    qsrc = bass.AP(
                        tensor=bpa.tensor,
                        offset=bpa.offset + n * BP_LEN + 1 + CH_OFF[h],
                        ap=[[1, K], [1, CH_W[h]]],
                    )
                    nc.gpsimd.dma_start(out=b_shc[:], in_=qsrc)
                    chunks.append(b_shc)
                b_shs.append(chunks)

            a_de = singles.tile([K, SW], BF16)
            nc.vector.tensor_copy(out=a_de[:], in_=x_v[:, 1, :])

            # norms in f32 from x_sb: per-partition partials (sample = p//32)
            scr_n = scr.tile([K, SW], F32, tag="scr_n")
            nc.vector.tensor_mul(scr_n[:], x_v[:, 1, :], x_v[:, 1, :])
            nc.vector.reduce_sum(stats[:, 4:5], scr_n[:], axis=mybir.AxisListType.X)
            scr_n2 = scr.tile([K, SW], F32, tag="scr_n")
            nc.vector.tensor_mul(scr_n2[:], x_v[:, 2, :], x_v[:, 2, :])
            nc.vector.reduce_sum(stats[:, 5:6], scr_n2[:], axis=mybir.AxisListType.X)

            # Transpose a_de halves once for ALL samples:
            # a_deT_*[tau, p] = a_de[p, tau (+128)] -> sample p//32, col block p%32
            a_te = psumt_pool.tile([K, K], BF16, tag="a_te")
            nc.tensor.transpose(a_te[:], a_de[:, 0:K], ident[:])
            a_to = psumt_pool.tile([K, K], BF16, tag="a_to")
            nc.tensor.transpose(a_to[:], a_de[:, K : 2 * K], ident[:])

            for n in range(NS):
                # A_cols: [64 zero | a bf16 | 64 zero]; even/odd g columns come
                # from the two transpose halves.  a_odd = A_cols shifted one
                # column so every matmul weight slice is 4-byte aligned.
                a_cols = acols_pool.tile([K, A_W], FP8, tag="a_cols")
                nc.vector.memset(a_cols[:], 0.0)
                av = a_cols[:].rearrange("p (g two) -> p two g", two=2)
                nc.vector.tensor_copy(out=av[:, 0, 32:64], in_=a_te[:, 32 * n : 32 * n + 32])
                nc.vector.tensor_copy(out=av[:, 1, 32:64], in_=a_to[:, 32 * n : 32 * n + 32])
                # 3 column-shifted copies so every weight slice is 4B-aligned
                a_phs = [a_cols]
                for r in range(1, 4):
                    a_ph = acols_pool.tile([K, A_W], FP8, tag=f"a_ph{r}", name=f"a_ph{r}")
                    nc.vector.tensor_copy(out=a_ph[:, 0 : A_W - r], in_=a_cols[:, r:A_W])
                    a_phs.append(a_ph)

                chunks = b_shs[n]

                # 65 accumulating matmuls; psum holds every corr lag once
                psum = psum_pool.tile([K, K], F32)
                for i in range(NT):
                    r = i % 4
                    lhsT = a_phs[r][:, i - r : i - r + K]
                    ch = min(i // 16, 3)
                    c0 = K * i - [0, 2048, 4096, 6144][ch]
                    nc.tensor.matmul(
                        psum[:],
                        lhsT,
                        chunks[ch][:, c0 : c0 + K],
                        start=(i == 0),
                        stop=(i == NT - 1),
                    )

                # sum(c^2) -> stats col n (square on ScalarE, reduce on DVE)
                scr_c2 = scr.tile([K, K], F32, tag="scr_c2")
                nc.scalar.activation(
                    out=scr_c2[:], in_=psum[:],
                    func=mybir.ActivationFunctionType.Square,
                )
                nc.vector.reduce_sum(
                    stats[:, n : n + 1], scr_c2[:], axis=mybir.AxisListType.X
                )

        if "bce" in parts:
            # ---- BCE: relu(x) - x*t + ln(1 + exp(-|x|)), per-class sums ----
            t_sb = bce_pool.tile([K, FW], F32)
            nc.sync.dma_start(
                out=t_sb[:],
                in_=targ.rearrange("n l c -> (n l c)").rearrange("(p f) -> p f", p=K),
            )
            ax = bce_pool.tile([K, FW], F32)
            nc.scalar.activation(ax[:], x_sb[:], mybir.ActivationFunctionType.Abs)
            ex = bce_pool.tile([K, FW], F32)
            nc.scalar.activation(
                ex[:], ax[:], mybir.ActivationFunctionType.Exp, scale=-1.0
            )
            sp = bce_pool.tile([K, FW], F32)
            nc.scalar.activation(
                sp[:], ex[:], mybir.ActivationFunctionType.Ln, bias=1.0
            )
            rx = bce_pool.tile([K, FW], F32)
            nc.vector.tensor_scalar_max(rx[:], x_sb[:], 0.0)
            xt = bce_pool.tile([K, FW], F32)
            nc.vector.tensor_mul(xt[:], x_sb[:], t_sb[:])
            v = bce_pool.tile([K, FW], F32)
            nc.vector.tensor_sub(v[:], rx[:], xt[:])
            nc.vector.tensor_add(v[:], v[:], sp[:])
            v_view = v[:].rearrange("p (t c) -> p c t", c=C)
            nc.vector.reduce_sum(
                stats[:, 6 : 6 + C], v_view, axis=mybir.AxisListType.X
            )

        nc.sync.dma_start(out=out[:], in_=stats[:])


def _build(parts=FULL_PARTS):
    global _CACHED_NC
    if _CACHED_NC is not None and _CACHED_NC[0] == parts:
        return _CACHED_NC[1]
    nc = bacc.Bacc(
        "TRN2",
        target_bir_lowering=False,
        debug=False,
        enable_asserts=False,
        num_devices=N_CORES,
    )
    with tile.TileContext(nc) as tc:
        _kernel_body(tc, parts)
    nc.compile()
    _CACHED_NC = (parts, nc)
    return nc


def host_reduce(stats_list, weight):
    """Final scalar reduction over per-core [128, 16] stats, in float64."""
    w = np.asarray(weight, dtype=np.float64)
    bce_sum = 0.0
    prox = 0.0
    for stats in stats_list:
        s = np.asarray(stats, dtype=np.float64)
        ss = s[:, 0:4].sum(axis=0)
        sa = s[:, 4].reshape(NS, 32).sum(axis=1)
        sb = s[:, 5].reshape(NS, 32).sum(axis=1)
        prox += float((ss / np.sqrt(sa * sb)).sum())
        bce_sum += float((s[:, 6:9].sum(axis=0) * w).sum())
    loss = LAMBDA1 * bce_sum / (N_FULL * L * C) + LAMBDA2 * prox
    return np.float32(loss)


def kernel(predictions, targets, weight, trace=False):
    global LAST_RESULT
    predictions = np.ascontiguousarray(np.asarray(predictions, dtype=np.float32))
    targets = np.ascontiguousarray(np.asarray(targets, dtype=np.float32))
    weight = np.asarray(weight, dtype=np.float32)
    assert predictions.shape == (N_FULL, L, C), predictions.shape

    nc = _build()
    in_maps = [
        {
            "predictions": np.ascontiguousarray(predictions[k * NS : (k + 1) * NS]),
            "targets": np.ascontiguousarray(targets[k * NS : (k + 1) * NS]),
        }
        for k in range(N_CORES)
    ]
    LAST_RESULT = run_bass_kernel_spmd(
        nc, in_maps, core_ids=list(range(N_CORES)), trace=trace
    )
    stats_list = [r["out"] for r in LAST_RESULT.results]
    return host_reduce(stats_list, weight)
